# revision 1
# baseline (speedup 1.0000x reference)
"""3D Gaussian blur (kernel_size=5, sigma=1.0) on (2,1,192,256,256) f32,
distributed over 8 Trainium2 NeuronCores.

The reference kernel factors exactly: g[i,j,l] = u[i]*v[l] (indep. of j),
so G = aD[i] * (1/5) * bW[l] and the 3D conv is separable into three 1D
convs: Gaussian along D, box along H, Gaussian along W.

Sharding: data-parallel over (batch, D-slab): 8 cores = 2 batches x 4 slabs
of 48 output slices each; each core receives its slab plus a 2-slice halo
(zero slices at batch edges), i.e. input [52, 256, 256].

Per-core kernel (Bass/Tile):
  pass A (per input slice): banded matmul out = X.T @ Bh fusing the H box
    conv with an H<->W transpose (h-major -> w-major), bf16.
  pass B (per output slice): 20 banded matmuls out += Y_s.T @ (aD_i * Bw)
    fusing the W Gaussian conv, the D Gaussian conv (via 5 scaled weight
    variants accumulated in PSUM across the 5 contributing slices), and the
    transpose back to h-major. PSUM accumulates in fp32.
Band matrices encode zero-padding at the edges natively.
"""
import numpy as np
import ml_dtypes

import concourse.bacc as bacc
import concourse.tile as tile
from concourse import mybir
from concourse.bass_utils import run_bass_kernel_spmd

B = 2          # batch
D = 192        # depth
HW = 256       # height = width
SLAB = 48      # output slices per core
DIN = SLAB + 4  # input slices per core (2-slice halo each side)
NB = 130       # band-split matmul N (128 + 2*2 halo)
P = 128
N_CORES = 8

F32 = mybir.dt.float32
BF16 = mybir.dt.bfloat16


def _taps():
    c = np.arange(5, dtype=np.float64) - 2
    u = np.exp(-c * c / 2.0)   # D-axis Gaussian (sigma=1)
    v = np.exp(-c * c)         # W-axis Gaussian (sigma^2=1/2)
    aD = (u / u.sum()).astype(np.float32)
    bW = (v / v.sum()).astype(np.float32)
    box = np.full(5, 0.2, dtype=np.float32)
    return aD, box, bW


def _band(rows, cols, roff, coff, taps):
    """M[r, c] = taps[(r+roff) - (c+coff) + 2] where |diff| <= 2, else 0."""
    m = np.zeros((rows, cols), dtype=np.float32)
    for r in range(rows):
        g = r + roff
        for c in range(cols):
            d = g - (c + coff)
            if -2 <= d <= 2:
                m[r, c] = taps[d + 2]
    return m


def _const_tensors():
    aD, box, bW = _taps()
    bh = np.stack([
        _band(P, NB, 0, 0, box),
        _band(P, NB, P, HW - NB, box),
    ])  # [2, 128, 130]
    bw = np.stack([
        np.stack([
            _band(P, NB, 0, 0, aD[i] * bW),
            _band(P, NB, P, HW - NB, aD[i] * bW),
        ])
        for i in range(5)
    ])  # [5, 2, 128, 130]
    return bh.astype(ml_dtypes.bfloat16), bw.astype(ml_dtypes.bfloat16)


def _build_nc():
    nc = bacc.Bacc("TRN2", target_bir_lowering=False, debug=False,
                   num_devices=N_CORES)
    # input pre-swizzled on host: x[p, s, hh, w] = slab[s, hh*128+p, w]
    x_d = nc.declare_dram_parameter("x", [P, DIN, 2, HW], BF16, isOutput=False)
    bh_d = nc.declare_dram_parameter("bh", [2, P, NB], BF16, isOutput=False)
    bw_d = nc.declare_dram_parameter("bw", [5, 2, P, NB], BF16, isOutput=False)
    # output swizzled: out[p, d, hb, w] = slice_d[hb*128+p, w]
    out_d = nc.declare_dram_parameter("out", [P, SLAB, 2, HW], F32, isOutput=True)

    XCHUNKS = [4, 12, 12, 12, 12]  # input slices per chunk DMA (sum = DIN)
    OCH = 4    # output slices per group DMA
    LAG = 3    # iterations between evac-A and pass-B consumption

    with tile.TileContext(nc) as tc:
        with (
            tc.tile_pool(name="consts", bufs=1) as cpool,
            tc.tile_pool(name="xbf", bufs=1) as xpool,
            tc.tile_pool(name="y", bufs=DIN // 2 + 1) as ypool,
            tc.tile_pool(name="osb", bufs=3) as opool,
            tc.tile_pool(name="pa", bufs=2, space="PSUM") as pa_pool,
            tc.tile_pool(name="pb", bufs=2, space="PSUM") as pb_pool,
        ):
            chunk_starts = []
            acc = 0
            for n in XCHUNKS:
                chunk_starts.append(acc)
                acc += n
            assert acc == DIN
            chunk_of = {}
            for ci, (st, n) in enumerate(zip(chunk_starts, XCHUNKS)):
                for s in range(st, st + n):
                    chunk_of[s] = (ci, s - st)

            bh_sb = cpool.tile([P, 2 * NB], BF16, tag="bh")
            bw_sb = cpool.tile([P, 10 * NB], BF16, tag="bw")

            # all input chunk DMAs dispatched up front on SP
            xchunks = []
            for ci, (st, n) in enumerate(zip(chunk_starts, XCHUNKS)):
                xc = xpool.tile([P, n, 2, HW], BF16, tag=f"xb{ci}")
                xchunks.append(xc)
                nc.sync.dma_start(xc[:], x_d[:, st:st + n])

            # const DMAs on ACT's hwdge queue
            nc.scalar.dma_start(bh_sb[:, 0:NB], bh_d[0])
            nc.scalar.dma_start(bh_sb[:, NB:2 * NB], bh_d[1])
            for i in range(5):
                for k in range(2):
                    j = i * 2 + k
                    nc.scalar.dma_start(bw_sb[:, j * NB:(j + 1) * NB], bw_d[i, k])

            ys2 = []

            def yv(s):
                return ys2[s // 2][:, s % 2]

            a_ps = None
            o_ps = None
            o_sb = None
            for it in range(DIN + 4 + LAG):
                s = it
                if s < DIN:
                    ci, sl = chunk_of[s]
                    x_b = xchunks[ci]
                    # pass A: H box conv + transpose -> w-major
                    # two slices share one 2-bank PSUM tile (1 bank each)
                    if s % 2 == 0:
                        a_ps = pa_pool.tile([P, 2, 2, HW], F32, tag="aps")
                    for wblk in range(2):
                        nc.tensor.matmul(
                            a_ps[:, s % 2, wblk, 0:NB],
                            x_b[:, sl, 0, wblk * P: wblk * P + P],
                            bh_sb[:, 0:NB],
                            start=wblk == 0, stop=False)
                        nc.tensor.matmul(
                            a_ps[:, s % 2, wblk, HW - NB:HW],
                            x_b[:, sl, 1, wblk * P: wblk * P + P],
                            bh_sb[:, NB:2 * NB],
                            start=False, stop=wblk == 1)
                    if s % 2 == 1:
                        y2 = ypool.tile([P, 2, 2, HW], BF16, tag="y")
                        ys2.append(y2)
                        nc.scalar.copy(y2[:], a_ps[:])

                dd = it - 4 - LAG  # output slice (inputs ready >= LAG iters ago)
                if not (0 <= dd < SLAB):
                    continue

                # pass B: W gauss conv (x aD tap) + transpose back
                # two outputs share one 2-bank PSUM tile (1 bank each)
                if dd % 2 == 0:
                    o_ps = pb_pool.tile([P, 2, 2, HW], F32, tag="ops")
                n_mm = 0
                for i in range(5):
                    ysrc = yv(dd + i)
                    for kh in range(2):
                        rhs = bw_sb[:, (i * 2 + kh) * NB:(i * 2 + kh + 1) * NB]
                        for hblk in range(2):
                            col0 = 0 if kh == 0 else HW - NB
                            nc.tensor.matmul(
                                o_ps[:, dd % 2, hblk, col0: col0 + NB],
                                ysrc[:, kh, hblk * P: hblk * P + P],
                                rhs,
                                start=n_mm == 0, stop=n_mm == 19)
                            n_mm += 1

                if dd % OCH == 0:
                    o_sb = opool.tile([P, OCH, 2, HW], F32, tag="osb")
                if dd % 2 == 1:
                    nc.vector.tensor_copy(
                        o_sb[:, dd % OCH - 1: dd % OCH + 1], o_ps[:])
                if dd % OCH == OCH - 1:
                    nc.sync.dma_start(
                        out_d[:, dd - OCH + 1: dd + 1], o_sb[:])

    nc.compile()
    return nc


_NC_CACHE = {}


def _get_nc():
    if "nc" not in _NC_CACHE:
        _NC_CACHE["nc"] = _build_nc()
    return _NC_CACHE["nc"]


def kernel(x, kernel_size, _trace=False, _trace_kwargs=None):
    """x: (2, 1, 192, 256, 256) float32; kernel_size: 5. Returns same shape."""
    assert int(kernel_size) == 5, "kernel hardcodes kernel_size=5"
    x = np.asarray(x)
    assert x.shape == (B, 1, D, HW, HW), x.shape
    in_dtype = x.dtype

    nc = _get_nc()
    bh, bw = _const_tensors()

    xp = np.zeros((B, D + 4, HW, HW), dtype=ml_dtypes.bfloat16)
    xp[:, 2:D + 2] = x[:, 0].astype(ml_dtypes.bfloat16)

    in_maps = []
    for c in range(N_CORES):
        b, j = divmod(c, 4)
        shard = xp[b, j * SLAB: j * SLAB + DIN]  # [52, 256, 256]
        sw = np.ascontiguousarray(
            shard.reshape(DIN, 2, P, HW).transpose(2, 0, 1, 3))
        in_maps.append({
            "x": sw,
            "bh": bh,
            "bw": bw,
        })

    res = run_bass_kernel_spmd(
        nc, in_maps, core_ids=list(range(N_CORES)),
        trace=_trace, **(_trace_kwargs or {}))

    out = np.empty((B, 1, D, HW, HW), dtype=np.float32)
    for c in range(N_CORES):
        b, j = divmod(c, 4)
        r = res.results[c]["out"]  # [128, 48, 2, 256]
        out[b, 0, j * SLAB:(j + 1) * SLAB] = (
            r.transpose(1, 2, 0, 3).reshape(SLAB, HW, HW))

    if _trace:
        kernel._last_result = res
    return out.astype(in_dtype, copy=False)

